# revision 11
# baseline (speedup 1.0000x reference)
"""Trainium2 Bass kernel for nn_LinearLayer_45243185496808.

Computes out[b,o] = sum_i tanh(x[b,i]*t) * (sum_p coef[o,i,p]) with
B=131072, I=O=128, P_NUM=16, data-parallel over batch on 8 NeuronCores.

Per-core pipeline (B_CORE=16384 rows):
  - prelude: w_T[i,o] = transpose(sum_p coef[o,i,p])  (resident in SBUF)
  - per 2048-row chunk (one contiguous 1 MiB DMA in / out):
      per 128-row slice: PE-transpose x -> PSUM, ScalarE tanh(scale*x)
      PSUM->SBUF, PE matmul against w_T -> PSUM, VectorE copy -> SBUF.
"""

import os
import sys
import types

import numpy as np

import concourse.bass as bass
import concourse.mybir as mybir
import concourse.tile as tile
from concourse import bacc, masks
from concourse.bass_utils import run_bass_kernel_spmd


def _ensure_ntff_hook():
    """Register the axon NTFF profile hook if the image lacks antenv.axon_hooks.

    Only needed for BASS_TRACE=1 profiling runs; harmless otherwise."""
    if "antenv.axon_hooks" in sys.modules:
        return
    try:
        from antenv.axon_hooks import get_axon_ntff_profile_hook  # noqa: F401

        return  # real module importable
    except ImportError:
        pass
    hook = None
    try:
        from trn_agent_boot.trn_boot import _ntff_profile_via_ctypes

        so_path = "/opt/axon/libaxon_pjrt.so"
        if os.path.exists(so_path):
            hook = _ntff_profile_via_ctypes(so_path)
    except Exception:
        hook = None
    mod = types.ModuleType("antenv.axon_hooks")
    mod.get_axon_ntff_profile_hook = lambda: hook
    mod.set_axon_ntff_profile_hook = lambda h: None
    sys.modules["antenv.axon_hooks"] = mod

N_CORES = 8
B_FULL = 131072
I_DIM = 128
O_DIM = 128
P_NUM = 16
P = 128              # SBUF partitions
RPP = 16             # rows of x per partition per chunk
CHUNK_ROWS = P * RPP        # 2048 rows -> 1 MiB per DMA
B_CORE = B_FULL // N_CORES  # 16384
N_CHUNKS = B_CORE // CHUNK_ROWS
G = 4                       # 128-row slices per PSUM-bank group

LAST_RESULT = None  # BassKernelResults of the most recent run (for test.py)


def build_bass(tanh_scale: float) -> bass.Bass:
    nc = bacc.Bacc("TRN2", target_bir_lowering=False)
    x = nc.dram_tensor("x", [B_CORE, I_DIM], mybir.dt.float32, kind="ExternalInput")
    coef = nc.dram_tensor(
        "coef", [O_DIM, I_DIM, P_NUM], mybir.dt.float32, kind="ExternalInput"
    )
    out = nc.dram_tensor("out", [B_CORE, O_DIM], mybir.dt.float32, kind="ExternalOutput")

    # Chunked views: partition p holds RPP consecutive rows, so each chunk is
    # one contiguous [128, RPP*128] f32 blob (8 KiB per partition).
    x_c = x[:, :].rearrange("(c p r) i -> c p (r i)", p=P, r=RPP)
    out_c = out[:, :].rearrange("(c p r) o -> c p (r o)", p=P, r=RPP)
    coef_flat = coef[:, :, :].rearrange("o i p -> o (i p)")

    with tile.TileContext(nc) as tc:
        with (
            tc.tile_pool(name="consts", bufs=1) as consts,
            tc.tile_pool(name="xin", bufs=3) as xin_pool,
            tc.tile_pool(name="vals", bufs=4) as vals_pool,
            tc.tile_pool(name="outp", bufs=3) as out_pool,
            tc.tile_pool(name="pxT", bufs=4, space="PSUM") as pxT_pool,
            tc.tile_pool(name="pout", bufs=4, space="PSUM") as pout_pool,
            tc.tile_pool(name="warm", bufs=1) as warm_pool,
        ):
            identity = consts.tile([P, P], mybir.dt.float32)
            masks.make_identity(nc, identity[:])

            # --- prelude: w_T = transpose(sum_p coef) ---
            coef_sb = consts.tile([P, I_DIM * P_NUM], mybir.dt.float32)
            nc.sync.dma_start(out=coef_sb[:], in_=coef_flat)
            coef3 = coef_sb[:].rearrange("o (i p) -> o i p", p=P_NUM)
            w_oi = consts.tile([P, I_DIM], mybir.dt.float32)
            nc.vector.tensor_copy(w_oi[:], coef3[:, :, 0])
            for p_idx in range(1, P_NUM):
                nc.vector.tensor_add(w_oi[:], w_oi[:], coef3[:, :, p_idx])
            w_psum = pxT_pool.tile([P, O_DIM], mybir.dt.float32, tag="xT_ps")
            nc.tensor.transpose(w_psum[:], w_oi[:], identity[:])
            w_T = consts.tile([P, O_DIM], mybir.dt.float32)
            nc.scalar.copy(w_T[:], w_psum[:])

            # PE warmup: ~5us of dense matmuls on the identity so HAM reaches
            # K=8/8 while the first chunk's DMA is still in flight.
            warm_sb = warm_pool.tile([P, P], mybir.dt.float32)
            nc.vector.tensor_copy(warm_sb[:], identity[:])
            for wi in range(4):
                wm_ps = pout_pool.tile([P, G * O_DIM], mybir.dt.float32, tag="o_ps")
                for wj in range(G):
                    nc.tensor.matmul(
                        wm_ps[:, wj * P : (wj + 1) * P],
                        warm_sb[:],
                        warm_sb[:],
                        start=True,
                        stop=True,
                    )

            # --- main loop: groups of G=4 slices share one PSUM bank ---
            for c in range(N_CHUNKS):
                x_sb = xin_pool.tile([P, RPP * I_DIM], mybir.dt.float32)
                nc.sync.dma_start(out=x_sb[:], in_=x_c[c])
                out_sb = out_pool.tile([P, RPP * O_DIM], mybir.dt.float32)
                for g in range(RPP // G):
                    xT_ps = pxT_pool.tile([P, G * P], mybir.dt.float32)
                    for j in range(G):
                        n = g * G + j
                        nc.tensor.transpose(
                            xT_ps[:, j * P : (j + 1) * P],
                            x_sb[:, n * I_DIM : (n + 1) * I_DIM],
                            identity[:],
                        )
                    v_T = vals_pool.tile([P, G * P], mybir.dt.float32)
                    nc.scalar.activation(
                        v_T[:],
                        xT_ps[:],
                        mybir.ActivationFunctionType.Tanh,
                        scale=tanh_scale,
                    )
                    o_ps = pout_pool.tile([P, G * O_DIM], mybir.dt.float32)
                    for j in range(G):
                        nc.tensor.matmul(
                            o_ps[:, j * O_DIM : (j + 1) * O_DIM],
                            v_T[:, j * P : (j + 1) * P],
                            w_T[:],
                            start=True,
                            stop=True,
                        )
                    nc.vector.tensor_copy(
                        out_sb[:, g * G * O_DIM : (g + 1) * G * O_DIM], o_ps[:]
                    )
                nc.sync.dma_start(out=out_c[c], in_=out_sb[:])
    nc.finalize()
    return nc


def kernel(x, coef, tanh_range):
    global LAST_RESULT
    x = np.ascontiguousarray(np.asarray(x, dtype=np.float32))
    coef = np.ascontiguousarray(np.asarray(coef, dtype=np.float32))
    t = float(np.asarray(tanh_range))
    assert x.shape == (B_FULL, I_DIM), x.shape
    assert coef.shape == (O_DIM, I_DIM, P_NUM), coef.shape

    nc = build_bass(t)
    in_maps = [
        {"x": np.ascontiguousarray(x[k * B_CORE : (k + 1) * B_CORE]), "coef": coef}
        for k in range(N_CORES)
    ]
    if os.environ.get("BASS_TRACE"):
        _ensure_ntff_hook()
    res = run_bass_kernel_spmd(nc, in_maps, core_ids=list(range(N_CORES)))
    LAST_RESULT = res
    return np.concatenate([r["out"] for r in res.results], axis=0)
